# revision 2
# baseline (speedup 1.0000x reference)
import sys

for _p in ("/opt/trn_rl_repo", "/root/.axon_site/_ro/trn_rl_repo"):
    if _p not in sys.path:
        sys.path.append(_p)

import numpy as np

# Problem: B=8 batches of cross-attention-like softmax matmul, one batch per core.
#   S[e,t] = sum_d enc[e,d] * dec[t,d]
#   A = softmax(S, axis=t)
#   C[t,d] = sum_e A[e,t] * enc[e,d]
B, S, D = 8, 2048, 1024
P = 128
EB = S // P   # 16 e-blocks
TB = S // P   # 16 t-blocks
DC = D // P   # 8 d-chunks (contraction for scores)
TC = S // 512 # 4 t-chunks of 512 (matmul free-dim limit)

_NC_CACHE = None


def _build():
    import concourse.bacc as bacc
    import concourse.tile as tile
    from concourse import mybir

    F32 = mybir.dt.float32
    F16 = mybir.dt.float16

    nc = bacc.Bacc("TRN2", target_bir_lowering=False, debug=False, num_devices=B)
    enc = nc.declare_dram_parameter("enc_outputs", [S, D], F32, isOutput=False)
    dec = nc.declare_dram_parameter("dec_outputs", [S, D], F32, isOutput=False)
    out = nc.declare_dram_parameter("out", [S, D], F32, isOutput=True)

    with tile.TileContext(nc) as tc:
        with (
            tc.tile_pool(name="dram16", bufs=1, space="DRAM") as dram_pool,
            tc.tile_pool(name="encT", bufs=1) as encT_pool,
            tc.tile_pool(name="decT", bufs=1) as decT_pool,
            tc.tile_pool(name="encn", bufs=1) as encn_pool,
            tc.tile_pool(name="pmat", bufs=1) as p_pool,
            tc.tile_pool(name="stats", bufs=4) as stats_pool,
            tc.tile_pool(name="ostage", bufs=3) as out_pool,
        ):
            enc16 = dram_pool.tile([S, D], F16, name="enc16")
            dec16 = dram_pool.tile([S, D], F16, name="dec16")

            # Phase A: f32 -> fp16 cast into DRAM scratch (SWDGE cast-DMA),
            # in halves so downstream transposes can start early.
            H = S // 2
            for h in range(2):
                nc.gpsimd.dma_start(
                    out=dec16[h * H : (h + 1) * H, :], in_=dec[h * H : (h + 1) * H, :]
                )
                nc.gpsimd.dma_start(
                    out=enc16[h * H : (h + 1) * H, :], in_=enc[h * H : (h + 1) * H, :]
                )

            decT = [decT_pool.tile([P, S], F16, name=f"decT{d}") for d in range(DC)]
            encT = [encT_pool.tile([P, S], F16, name=f"encT{d}") for d in range(DC)]
            encn = [encn_pool.tile([P, D], F16, name=f"encn{e}") for e in range(EB)]

            # Transposed loads: decT[d][dd, t] = dec16[t, d*P+dd] (xbar transpose DMA)
            for d in range(DC):
                for h in range(2):
                    nc.sync.dma_start(
                        out=decT[d][:, h * H : (h + 1) * H],
                        in_=dec16[h * H : (h + 1) * H, d * P : (d + 1) * P],
                        transpose=True,
                    )
                    nc.sync.dma_start(
                        out=encT[d][:, h * H : (h + 1) * H],
                        in_=enc16[h * H : (h + 1) * H, d * P : (d + 1) * P],
                        transpose=True,
                    )
            for e in range(EB):
                nc.sync.dma_start(
                    out=encn[e][:], in_=enc16[e * P : (e + 1) * P, :]
                )

            pmat = [p_pool.tile([P, S], F16, name=f"p{e}") for e in range(EB)]

            # Phase B: scores + softmax per e-block.
            with tc.tile_pool(name="psum_s", bufs=2, space="PSUM") as psum_s:
                for e in range(EB):
                    s_ps = psum_s.tile([P, S], F32, name="s_ps")
                    for d in range(DC):
                        for t in range(TC):
                            nc.tensor.matmul(
                                s_ps[:, t * 512 : (t + 1) * 512],
                                lhsT=encT[d][:, e * P : (e + 1) * P],
                                rhs=decT[d][:, t * 512 : (t + 1) * 512],
                                start=(d == 0),
                                stop=(d == DC - 1),
                            )
                    negmax = stats_pool.tile([P, 1], F32, name="negmax")
                    nc.vector.reduce_max(
                        out=negmax, in_=s_ps[:], axis=mybir.AxisListType.X, negate=True
                    )
                    z = stats_pool.tile([P, 1], F32, name="z")
                    nc.scalar.activation(
                        out=pmat[e][:],
                        in_=s_ps[:],
                        func=mybir.ActivationFunctionType.Exp,
                        bias=negmax,
                        scale=1.0,
                        accum_out=z,
                    )
                    zinv = stats_pool.tile([P, 1], F32, name="zinv")
                    nc.vector.reciprocal(zinv, z)
                    # encn[e] <- enc[e] / Z[e]  (per-partition scalar, fp16 out)
                    nc.vector.tensor_scalar_mul(encn[e][:], encn[e][:], zinv)

            # Phase C: context C[t,:] = sum_e P[e,t] * encZ[e,:]
            with tc.tile_pool(name="psum_c", bufs=2, space="PSUM") as psum_c:
                for t in range(TB):
                    c_ps = psum_c.tile([P, D], F32, name="c_ps")
                    for e in range(EB):
                        for hf in range(2):
                            nc.tensor.matmul(
                                c_ps[:, hf * 512 : (hf + 1) * 512],
                                lhsT=pmat[e][:, t * P : (t + 1) * P],
                                rhs=encn[e][:, hf * 512 : (hf + 1) * 512],
                                start=(e == 0),
                                stop=(e == EB - 1),
                            )
                    o_t = out_pool.tile([P, D], F32, name="o_t")
                    nc.any.tensor_copy(out=o_t[:], in_=c_ps[:])
                    nc.scalar.dma_start(out=out[t * P : (t + 1) * P, :], in_=o_t[:])

    nc.compile()
    return nc


def _get_nc():
    global _NC_CACHE
    if _NC_CACHE is None:
        _NC_CACHE = _build()
    return _NC_CACHE


def kernel(enc_outputs, dec_outputs, _want_results=False, **_ignored):
    from concourse.bass_utils import run_bass_kernel_spmd

    nc = _get_nc()
    enc_outputs = np.asarray(enc_outputs, dtype=np.float32)
    dec_outputs = np.asarray(dec_outputs, dtype=np.float32)
    in_maps = [
        {
            "enc_outputs": np.ascontiguousarray(enc_outputs[b]),
            "dec_outputs": np.ascontiguousarray(dec_outputs[b]),
        }
        for b in range(B)
    ]
    res = run_bass_kernel_spmd(nc, in_maps, core_ids=list(range(B)))
    out = np.stack([res.results[b]["out"] for b in range(B)], axis=0)
    if _want_results:
        return out, res
    return out
